# revision 1
# baseline (speedup 1.0000x reference)
"""Causal attention on 8 TRN2 cores — transposed-scores variant (v2).

2 cores per batch; the PAIR splits the KEYS: parity c owns key blocks
S_c = sorted({2p+c} u {31-2p-c}) (16 of 32 blocks), projects K^T/V for
its 2048 keys + Q^T for all 4096 queries, computes partial unnormalized
attention, host combines y = (num0+num1)/(ell0+ell1).

v2 computes scores TRANSPOSED: S_T[k, q] = (KT_i-block stationary) @
(QT_i-chunk moving), so exp(S_T) is directly the stationary operand of
the attn@V matmul — no PE transposes, no PSUM->SBUF prob copies, and
ell comes from tiny P_T @ ones matmuls. Queries are processed in chunks
of 4 blocks (512 q) so the scores matmul keeps N=512; Q projection is
interleaved chunk-by-chunk so only 512 q-columns of QT live in SBUF.

Program structure is parity-independent (one SPMD NEFF); parity enters
only through data: which keys are in xT, and the 2 diagonal-band masks
per chunk (keyset blocks 2c and 2c+1) that cut beyond-causal entries.
"""

import math
import sys

sys.path.insert(0, "/opt/trn_rl_repo")

import ml_dtypes
import numpy as np

import concourse.mybir as mybir
import concourse.tile as tile
from concourse import bacc
from concourse.bass_utils import run_bass_kernel_spmd

B = 4
S = 4096
D = 1024
P = 128
DC = D // P          # 8 chunks of the contraction dim
NQB = 32             # query blocks per batch
NCH = 8              # query chunks (4 blocks = 512 q each)
HALF = S // 2        # keys owned per core
BF16 = mybir.dt.bfloat16
F32 = mybir.dt.float32
NEG = -1.0e9
SCALE = 1.0 / math.sqrt(D)


def _keyset(c):
    return sorted({2 * p + c for p in range(8)} | {31 - 2 * p - c for p in range(8)})


def _L(g):
    return (g + 2) // 2  # ceil((g+1)/2): unified per-parity kv-block count


def _build_program(reps=1):
    nc = bacc.Bacc("TRN2", target_bir_lowering=False, debug=False)

    xT = nc.dram_tensor("xT", [D, HALF], BF16, kind="ExternalInput").ap()
    xTq = nc.dram_tensor("xTq", [D, S], BF16, kind="ExternalInput").ap()
    # mT = (Wq @ Wk.T).T — the QK bilinear form folded into one matrix, so
    # only keys get projected (K' = x_k @ M.T) and raw x serves as queries
    mT = nc.dram_tensor("mT", [D, D], BF16, kind="ExternalInput").ap()
    wv = nc.dram_tensor("wv", [D, D], BF16, kind="ExternalInput").ap()
    mask = nc.dram_tensor("mask", [2 * NCH, P, 512], F32, kind="ExternalInput").ap()
    y = nc.dram_tensor("y", [S, D], BF16, kind="ExternalOutput").ap()
    ell = nc.dram_tensor("ell", [1, S], F32, kind="ExternalOutput").ap()

    with tile.TileContext(nc) as tc:
        with (
            tc.tile_pool(name="big", bufs=1) as big,
            tc.tile_pool(name="wpool", bufs=2) as wpool,
            tc.tile_pool(name="xslab", bufs=2) as xslab,
            tc.tile_pool(name="qt", bufs=2) as qt_pool,
            tc.tile_pool(name="mk", bufs=2) as mk_pool,
            tc.tile_pool(name="pT", bufs=2) as pT_pool,
            tc.tile_pool(name="yy", bufs=2) as y_pool,
            tc.tile_pool(name="els", bufs=2) as els_pool,
            tc.tile_pool(name="ps", bufs=3, space="PSUM") as ps,
            tc.tile_pool(name="yp", bufs=4, space="PSUM") as yp_pool,
            tc.tile_pool(name="elp", bufs=1, space="PSUM") as el_pool,
        ):
          for _rep in range(reps):
            KT = big.tile([P, DC, HALF], BF16, tag="KT")     # 32 KB/part
            V = big.tile([P, 16, D], BF16, tag="V")          # 32 KB/part
            ones = big.tile([P, 1], BF16, tag="ones")
            nc.gpsimd.memset(ones[:], 1.0)

            # ---- fused K'^T + V projection over the local key half -----
            # K'^T[a, key] = sum_b M[a, b] xT[b, key]: identical matmul
            # structure to a plain K projection, with mT in place of Wk.
            wk_t = wpool.tile([P, DC, D], BF16, tag="W")
            for _i in range(DC):
                nc.scalar.dma_start(
                    out=wk_t[:, _i, :], in_=mT[_i * P : (_i + 1) * P, :]
                )
            wv_t = wpool.tile([P, DC, D], BF16, tag="W")
            for _i in range(DC):
                nc.scalar.dma_start(
                    out=wv_t[:, _i, :], in_=wv[_i * P : (_i + 1) * P, :]
                )
            for kt in range(4):  # local key tiles of 512
                xs = xslab.tile([P, DC, 512], BF16, tag="xs")
                for _i in range(DC):
                    nc.sync.dma_start(
                        out=xs[:, _i, :],
                        in_=xT[_i * P : (_i + 1) * P,
                               kt * 512 : (kt + 1) * 512],
                    )
                for j in range(DC):
                    pt = ps.tile([P, 512], F32, tag="ps", name=f"kp{kt}_{j}")
                    for i in range(DC):
                        nc.tensor.matmul(
                            pt[:],
                            lhsT=wk_t[:, i, j * P : (j + 1) * P],
                            rhs=xs[:, i, :],
                            start=(i == 0),
                            stop=(i == DC - 1),
                        )
                    nc.vector.tensor_copy(
                        KT[:, j, kt * 512 : (kt + 1) * 512], pt[:]
                    )
                for sb in range(4):
                    kb = kt * 4 + sb
                    pv = [ps.tile([P, 512], F32, tag="ps", name=f"v{n}_{kb}")
                          for n in range(2)]
                    for i in range(DC):
                        for n in range(2):
                            nc.tensor.matmul(
                                pv[n][:],
                                lhsT=xs[:, i, sb * P : (sb + 1) * P],
                                rhs=wv_t[:, i, n * 512 : (n + 1) * 512],
                                start=(i == 0),
                                stop=(i == DC - 1),
                            )
                    for n in range(2):
                        nc.scalar.copy(V[:, kb, n * 512 : (n + 1) * 512], pv[n][:])

            # ---- per query chunk: raw x as queries, scores, attn@V -----
            for c in range(NCH):
                Lmax = 2 * c + 2

                # queries are raw x columns (the W_q W_k^T fold lives in K')
                QT = qt_pool.tile([P, DC, 512], BF16, tag="QT")
                for _i in range(DC):
                    nc.sync.dma_start(
                        out=QT[:, _i, :],
                        in_=xTq[_i * P : (_i + 1) * P,
                                c * 512 : (c + 1) * 512],
                    )

                # masks for the two diagonal-band key blocks (kbi 2c, 2c+1)
                mks = []
                for sl in range(2):
                    mk = mk_pool.tile([P, 512], F32, tag="mk",
                                      name=f"mk{c}_{sl}")
                    nc.scalar.dma_start(out=mk[:], in_=mask[2 * c + sl])
                    mks.append(mk)

                # transposed scores S_T[k, q] + exp -> P_T, per local kv blk.
                # ell[q] = sum_k P_T[k, q] accumulates via a ones-stationary
                # matmul per block, lagged one block behind the scores so the
                # PE never waits on the exp that produces its rhs.
                pT = pT_pool.tile([P, 16, 512], BF16, tag="pT")
                elps = el_pool.tile([1, 512], F32, tag="elp", name=f"elp{c}")

                def ell_mm(kbi):
                    nc.tensor.matmul(
                        elps[0:1, :],
                        lhsT=ones[:, 0:1],
                        rhs=pT[:, kbi, :],
                        start=(kbi == 0),
                        stop=(kbi == Lmax - 1),
                    )

                for kbi in range(Lmax):
                    pts = ps.tile([P, 512], F32, tag="ps", name=f"sc{c}_{kbi}")
                    for i in range(DC):
                        nc.tensor.matmul(
                            pts[:],
                            lhsT=KT[:, i, kbi * P : (kbi + 1) * P],
                            rhs=QT[:, i, :],
                            start=(i == 0),
                            stop=(i == DC - 1),
                        )
                    if kbi >= 2 * c:
                        nc.vector.tensor_add(pts[:], pts[:], mks[kbi - 2 * c][:])
                    nc.scalar.activation(
                        pT[:, kbi, :],
                        pts[:],
                        mybir.ActivationFunctionType.Exp,
                        bias=0.0,
                        scale=SCALE,
                    )
                    if kbi > 0:
                        ell_mm(kbi - 1)

                # attn @ V, per query block of the chunk
                for gi in range(4):
                    g = 4 * c + gi
                    Lg = _L(g)
                    yps = [yp_pool.tile([P, 512], F32, tag="yp",
                                        name=f"y{n}_{g}")
                           for n in range(2)]
                    for kbi in range(Lg):
                        pslab = pT[:, kbi, gi * P : (gi + 1) * P]
                        for n in range(2):
                            nc.tensor.matmul(
                                yps[n][:],
                                lhsT=pslab,
                                rhs=V[:, kbi, n * 512 : (n + 1) * 512],
                                start=(kbi == 0),
                                stop=(kbi == Lg - 1),
                            )
                    if gi == 0:
                        ell_mm(Lmax - 1)
                        els = els_pool.tile([1, 512], F32, tag="els")
                        nc.vector.tensor_copy(els[0:1, :], elps[0:1, :])
                        nc.gpsimd.dma_start(
                            out=ell[0:1, c * 512 : (c + 1) * 512], in_=els[0:1, :]
                        )
                    # evacuate numerator: n=0 on scalar, n=1 on vector so
                    # neither engine bottlenecks the early (small-L) chunks
                    ys0 = y_pool.tile([P, 512], BF16, tag="y")
                    nc.scalar.copy(ys0[:], yps[0][:])
                    nc.gpsimd.dma_start(
                        out=y[g * P : (g + 1) * P, 0:512], in_=ys0[:]
                    )
                    ys1 = y_pool.tile([P, 512], BF16, tag="y")
                    nc.vector.tensor_copy(ys1[:], yps[1][:])
                    nc.gpsimd.dma_start(
                        out=y[g * P : (g + 1) * P, 512:1024], in_=ys1[:]
                    )
    nc.finalize()
    return nc


_NC = None


def _get_program():
    global _NC
    if _NC is None:
        _NC = _build_program()
    return _NC


def _build_mask(c):
    """mask[2*ch+sl, p, qcol]: additive mask for key block ks[2*ch+sl]
    against query chunk ch (global q = ch*512 + qcol, k = b*128 + p)."""
    ks = _keyset(c)
    m = np.zeros((2 * NCH, P, 512), np.float32)
    q = np.arange(512)[None, :]
    p = np.arange(P)[:, None]
    for ch in range(NCH):
        for sl in range(2):
            b = ks[2 * ch + sl]
            keep = (b * P + p) <= (ch * 512 + q)
            m[2 * ch + sl] = np.where(keep, 0.0, NEG)
    return m


def _make_in_maps(x, Wq, Wk, Wv, cores=range(8)):
    bf = ml_dtypes.bfloat16
    # fold the QK bilinear form on the host: scores = x_q (Wq Wk^T) x_k^T
    m = np.asarray(Wq, np.float32) @ np.asarray(Wk, np.float32).T
    mTb = np.ascontiguousarray(m.T.astype(bf))
    wvb = np.ascontiguousarray(Wv.astype(bf))
    masks = [_build_mask(0), _build_mask(1)]
    keycols = [
        np.concatenate([np.arange(b * P, (b + 1) * P) for b in _keyset(c)])
        for c in (0, 1)
    ]

    in_maps = []
    for core in cores:
        b, c = core // 2, core % 2
        xb = x[b]
        in_maps.append(
            {
                "xT": np.ascontiguousarray(xb[keycols[c]].T.astype(bf)),
                "xTq": np.ascontiguousarray(xb.T.astype(bf)),
                "mT": mTb,
                "wv": wvb,
                "mask": masks[c],
            }
        )
    return in_maps


def kernel(x, Wq, Wk, Wv):
    nc = _get_program()
    in_maps = _make_in_maps(x, Wq, Wk, Wv)

    res = run_bass_kernel_spmd(nc, in_maps, core_ids=list(range(8))).results

    out = np.empty((B, S, D), np.float32)
    for b in range(B):
        r0, r1 = res[2 * b], res[2 * b + 1]
        num = r0["y"].astype(np.float32) + r1["y"].astype(np.float32)
        l0 = r0["ell"].reshape(S, 1)
        l1 = r1["ell"].reshape(S, 1)
        out[b] = num / (l0 + l1)
    return out

